# revision 1
# baseline (speedup 1.0000x reference)
"""Channel-attention (bmm-softmax-bmm over channels) on 8 TRN2 NeuronCores.

Math (per batch b):
    q = Wq x + bq 1^T ; k = Wk x + bk 1^T ; v = Wv x + bv 1^T      (x: [C, P])
    E = q k^T ; attn = softmax(E, axis=-1) ; out = attn v

Gram reformulation (cuts MACs ~2.1x):
    G = x x^T (symmetric: only upper-triangle block-row computed)
    s = x @ 1_P
    E = WqT.T @ (G WkT + s bk^T) + bq r^T,   r = Wk s + P bk
    attn_un = exp(E - rowmax), Z = rowsum(attn_un)
    AT = Wv^T attn_un^T ;  t = attn_un @ bv
    out = (AT.T @ x + t 1^T) * (1/Z) per-row

Sharding: data-parallel over B; core i gets batches [2i, 2i+1]; weights
replicated. No cross-core communication.  Compute dtype float32r
(TF32-like matmul at 4x the fp32 rate; measured end-to-end rel err
~1.7e-3 vs fp32 reference).
"""

import os
from contextlib import ExitStack

import numpy as np

import concourse.bass as bass
from concourse import bacc
import concourse.mybir as mybir
import concourse.tile as tile
from concourse.bass_utils import run_bass_kernel_spmd

B, C, P = 16, 512, 4096
N_CORES = 8
BPC = B // N_CORES           # batches per core
CT = C // 128                # 4 c-tiles
QTR = P // 4                 # 1024-wide x quarters
NQ = 4                       # quarters per batch
QT_Q = QTR // 128            # 8 p-tiles per quarter
PBQ = QTR // 512             # 2 512-wide out chunks per quarter
F32 = mybir.dt.float32
F32R = mybir.dt.float32r

AX = mybir.AxisListType
ALU = mybir.AluOpType
ACTF = mybir.ActivationFunctionType


def _dt(name, default):
    v = os.environ.get(name, default)
    return {"f32": F32, "f32r": F32R}[v]


def build_nc(st_dt=None):
    # Storage dtype of every matmul-feeding tensor. walrus requires fp32r
    # matmult operands to be *produced* as float32r, so the dtype lives on
    # the tiles/DRAM tensors rather than on per-matmul bitcasts.
    ST = st_dt or _dt("KDT", "f32r")

    nc = bacc.Bacc(trn_type="TRN2", target_bir_lowering=False, debug=False)

    x_d = nc.dram_tensor("x", [BPC, C, P], ST, kind="ExternalInput")
    wqt_d = nc.dram_tensor("wqt", [C, C], ST, kind="ExternalInput")
    wkt_d = nc.dram_tensor("wkt", [C, C], ST, kind="ExternalInput")
    wv_d = nc.dram_tensor("wv", [C, C], ST, kind="ExternalInput")
    bqr_d = nc.dram_tensor("bq_row", [1, C], ST, kind="ExternalInput")
    bkr_d = nc.dram_tensor("bk_row", [1, C], ST, kind="ExternalInput")
    pbk_d = nc.dram_tensor("pbk_row", [1, C], ST, kind="ExternalInput")
    bvr_d = nc.dram_tensor("bv_row", [1, C], ST, kind="ExternalInput")
    ident_d = nc.dram_tensor("ident", [128, 128], ST, kind="ExternalInput")
    out_d = nc.dram_tensor("out", [BPC, C, P], F32, kind="ExternalOutput")

    with ExitStack() as ctx:
        tc = ctx.enter_context(tile.TileContext(nc))
        const = ctx.enter_context(tc.tile_pool(name="const", bufs=1))
        xpool = ctx.enter_context(tc.tile_pool(name="xpool", bufs=6))
        midp = ctx.enter_context(tc.tile_pool(name="midp", bufs=4))
        xtp = ctx.enter_context(tc.tile_pool(name="xtp", bufs=3))
        vecp = ctx.enter_context(tc.tile_pool(name="vecp", bufs=2))
        outp = ctx.enter_context(tc.tile_pool(name="outp", bufs=2))
        gps = ctx.enter_context(tc.tile_pool(name="gps", bufs=1, space="PSUM"))
        mmps = ctx.enter_context(tc.tile_pool(name="mmps", bufs=2, space="PSUM"))
        ops = ctx.enter_context(tc.tile_pool(name="ops", bufs=2, space="PSUM"))

        # ---- identity + x loads first: PE's first transposes need only
        # ident and x quarter 0 ----
        ident = const.tile([128, 128], ST, name="ident")
        nc.sync.dma_start(out=ident, in_=ident_d[:, :])
        st0, st1 = {}, {}

        def load_x(b, q, st, split=1):
            xt = xpool.tile([128, CT, QTR], ST, name=f"x_b{b}q{q}", tag="x")
            w = QTR // split
            for s in range(split):
                nc.sync.dma_start(
                    out=xt[:, :, s * w : (s + 1) * w],
                    in_=x_d[
                        b, :, q * QTR + s * w : q * QTR + (s + 1) * w
                    ].rearrange("(t p) f -> p t f", p=128),
                )
            st[f"x{q}"] = xt

        load_x(0, 0, st0, split=4)
        for q in range(1, NQ):
            load_x(0, q, st0)
        load_x(1, 0, st1)
        load_x(1, 1, st1)

        # ---- constants (DMA-queued behind the critical x loads) ----
        # memset rejects float32r; build in f32 and cast-copy (1.0 is exact).
        ones11_f = const.tile([1, 1], F32, name="ones11_f")
        nc.vector.memset(ones11_f, 1.0)
        if ST is F32:
            ones11 = ones11_f
        else:
            ones11 = const.tile([1, 1], ST, name="ones11")
            nc.vector.tensor_copy(ones11, ones11_f)

        def load_w(name, d):
            t = const.tile([128, CT, C], ST, name=name)
            nc.sync.dma_start(out=t, in_=d[:, :].rearrange("(t p) f -> p t f", p=128))
            return t

        wkt_sb = load_w("wkt_sb", wkt_d)   # needed first (U phase)
        wqt_sb = load_w("wqt_sb", wqt_d)
        wv_sb = load_w("wv_sb", wv_d)
        bqr_sb = const.tile([1, C], ST, name="bqr_sb")
        nc.sync.dma_start(out=bqr_sb, in_=bqr_d[:, :])
        bkr_sb = const.tile([1, C], ST, name="bkr_sb")
        nc.sync.dma_start(out=bkr_sb, in_=bkr_d[:, :])
        pbk_sb = const.tile([1, C], ST, name="pbk_sb")
        nc.sync.dma_start(out=pbk_sb, in_=pbk_d[:, :])
        bv_rep = const.tile([128, C], ST, name="bv_rep")
        nc.sync.dma_start(out=bv_rep, in_=bvr_d[:, :].partition_broadcast(128))

        def copy_evac(i, out, in_):
            # alternate evacuation engine to balance DVE/ACT load
            if i % 2 == 0:
                nc.scalar.copy(out, in_)
            else:
                nc.vector.tensor_copy(out, in_)

        # ---- per-batch phases ----
        sdump = const.tile([128, QTR], F32, name="sdump")

        def sreduce(b, q, st):
            # row-sums on the Scalar engine: Identity activation with
            # accumulate output; the main output goes to a shared scratch.
            sc = vecp.tile([128, CT], F32, name=f"scol_q{q}", tag=f"scol_q{q}")
            for t in range(CT):
                nc.scalar.activation(
                    out=sdump,
                    in_=st[f"x{q}"][:, t, :].bitcast(F32),
                    func=ACTF.Identity,
                    accum_out=sc[:, t : t + 1],
                )
            st[f"scol_q{q}"] = sc

        def transpose_G(b, q, st):
            """Per p-tile: PE-transpose x -> xT (psum), copy to SBUF, then
            upper-triangle accumulating G matmuls.  G-matmuls for p-tile k
            are emitted after the transposes of p-tile k+1 so the
            PSUM->SBUF copy latency stays off the PE critical path."""
            if q == 0:
                st["G_ps"] = [
                    gps.tile([128, C - cc * 128], F32, name=f"G{cc}_b{b}", tag=f"G{cc}")
                    for cc in range(CT)
                ]

            def emit_G(xT_sb, first, last):
                for cc in range(CT):
                    nc.tensor.matmul(
                        out=st["G_ps"][cc],
                        lhsT=xT_sb[:, cc * 128 : (cc + 1) * 128],
                        rhs=xT_sb[:, cc * 128 :],
                        start=first,
                        stop=last,
                    )

            for k in range(QT_Q):
                xT_ps = mmps.tile([128, C], ST, name="xT_ps", tag="mm")
                for t in range(CT):
                    nc.tensor.transpose(
                        out=xT_ps[:, t * 128 : (t + 1) * 128],
                        in_=st[f"x{q}"][:, t, k * 128 : (k + 1) * 128],
                        identity=ident,
                    )
                xT_sb = xtp.tile([128, C], ST, name="xT_sb", tag="xt")
                nc.vector.tensor_copy(xT_sb, xT_ps)
                if "pending_xt" in st:
                    p_sb, p_first = st.pop("pending_xt")
                    emit_G(p_sb, p_first, False)
                st["pending_xt"] = (xT_sb, q == 0 and k == 0)
            if q == NQ - 1:
                p_sb, p_first = st.pop("pending_xt")
                emit_G(p_sb, p_first, True)

        def s_finish(b, st):
            sa = vecp.tile([128, CT], F32, name="sa", tag="sa")
            sb_ = vecp.tile([128, CT], F32, name="sb_", tag="sb_")
            nc.vector.tensor_add(sa, st["scol_q0"], st["scol_q1"])
            nc.vector.tensor_add(sb_, st["scol_q2"], st["scol_q3"])
            scol = vecp.tile([128, CT], ST, name="scol", tag="scol")
            nc.vector.tensor_add(scol, sa, sb_)
            st["scol"] = scol
            srow_ps = mmps.tile([1, C], ST, name="srow_ps", tag="mm")
            for t in range(CT):
                nc.tensor.transpose(
                    out=srow_ps[:, t * 128 : (t + 1) * 128],
                    in_=scol[:, t : t + 1],
                    identity=ident,
                )
            srow = vecp.tile([1, C], ST, name="srow", tag="srow", bufs=1)
            nc.vector.tensor_copy(srow, srow_ps)
            st["srow"] = srow

        def G_copy(b, st):
            """Evacuate the upper-triangle block-row of G and mirror the
            strictly-lower blocks via PE transposes (G is symmetric)."""
            G_sb = midp.tile([128, CT, C], ST, name="G_sb", tag="mid")
            for cc in range(CT):
                copy_evac(cc, G_sb[:, cc, cc * 128 :], st["G_ps"][cc])
            # lower-triangle fill: G[cc, dd] = G[dd, cc]^T for dd < cc
            pairs = [(dd, cc) for cc in range(CT) for dd in range(cc)]
            lps = [mmps.tile([128, C], ST, name="lps", tag="mm") for _ in range(2)]
            for i, (dd, cc) in enumerate(pairs):
                nc.tensor.transpose(
                    out=lps[i // 4][:, (i % 4) * 128 : (i % 4 + 1) * 128],
                    in_=G_sb[:, dd, cc * 128 : (cc + 1) * 128],
                    identity=ident,
                )
            for i, (dd, cc) in enumerate(pairs):
                copy_evac(
                    i,
                    G_sb[:, cc, dd * 128 : (dd + 1) * 128],
                    lps[i // 4][:, (i % 4) * 128 : (i % 4 + 1) * 128],
                )
            st["G_sb"] = G_sb
            del st["G_ps"]

        def U_phase(b, st):
            U_sb = midp.tile([128, CT, C], ST, name="U_sb", tag="mid")
            for ic in range(CT):
                u_ps = ops.tile([128, C], F32, name="u_ps", tag="out")
                for e in range(CT):
                    nc.tensor.matmul(
                        out=u_ps,
                        lhsT=st["G_sb"][:, e, ic * 128 : (ic + 1) * 128],
                        rhs=wkt_sb[:, e, :],
                        start=(e == 0),
                        stop=False,
                    )
                nc.tensor.matmul(
                    out=u_ps,
                    lhsT=st["srow"][:, ic * 128 : (ic + 1) * 128],
                    rhs=bkr_sb,
                    start=False,
                    stop=True,
                )
                copy_evac(ic, U_sb[:, ic, :], u_ps)
            st["U_sb"] = U_sb
            # r = Wk s + P bk   (as a row [1, C])
            r_ps = mmps.tile([1, C], F32, name="r_ps", tag="mm")
            for t in range(CT):
                nc.tensor.matmul(
                    out=r_ps,
                    lhsT=st["scol"][:, t : t + 1],
                    rhs=wkt_sb[:, t, :],
                    start=(t == 0),
                    stop=False,
                )
            nc.tensor.matmul(
                out=r_ps, lhsT=ones11, rhs=pbk_sb, start=False, stop=True
            )
            rrow = vecp.tile([1, C], ST, name="rrow", tag="rrow", bufs=1)
            nc.vector.tensor_copy(rrow, r_ps)
            st["rrow"] = rrow

        def E_softmax(b, st):
            attn_sb = midp.tile([128, CT, C], ST, name="attn_sb", tag="mid")
            E_sb = midp.tile([128, CT, C], F32, name="E_sb", tag="esb", bufs=1)
            mx = vecp.tile([128, CT], F32, name="mx", tag="mx")
            negm = vecp.tile([128, CT], F32, name="negm", tag="negm")
            zsum = vecp.tile([128, CT], F32, name="zsum", tag="zsum")
            recip = vecp.tile([128, CT], F32, name="recip", tag="recip")
            for cc in range(CT):
                e_ps = ops.tile([128, C], F32, name="e_ps", tag="out")
                for i in range(CT):
                    nc.tensor.matmul(
                        out=e_ps,
                        lhsT=wqt_sb[:, i, cc * 128 : (cc + 1) * 128],
                        rhs=st["U_sb"][:, i, :],
                        start=(i == 0),
                        stop=False,
                    )
                nc.tensor.matmul(
                    out=e_ps,
                    lhsT=bqr_sb[:, cc * 128 : (cc + 1) * 128],
                    rhs=st["rrow"],
                    start=False,
                    stop=True,
                )
                # fast PSUM evacuation (frees the bank for batch overlap)
                copy_evac(cc, E_sb[:, cc, :], e_ps)
                nc.vector.reduce_max(
                    out=mx[:, cc : cc + 1], in_=E_sb[:, cc, :], axis=AX.X
                )
                nc.vector.tensor_scalar_mul(
                    negm[:, cc : cc + 1], mx[:, cc : cc + 1], -1.0
                )
                nc.scalar.activation(
                    out=attn_sb[:, cc, :],
                    in_=E_sb[:, cc, :],
                    func=ACTF.Exp,
                    bias=negm[:, cc : cc + 1],
                    scale=1.0,
                    accum_out=zsum[:, cc : cc + 1],
                )
            nc.vector.reciprocal(out=recip, in_=zsum)
            st["attn"] = attn_sb
            st["recip"] = recip

        def attnT_AT(b, st):
            attnT_sb = midp.tile([128, CT, C], ST, name="attnT_sb", tag="mid")
            for dc in range(CT):
                at_ps = mmps.tile([128, C], ST, name="at_ps", tag="mm")
                for t in range(CT):
                    nc.tensor.transpose(
                        out=at_ps[:, t * 128 : (t + 1) * 128],
                        in_=st["attn"][:, t, dc * 128 : (dc + 1) * 128],
                        identity=ident,
                    )
                copy_evac(dc, attnT_sb[:, dc, :], at_ps)
            AT_sb = midp.tile([128, CT, C], ST, name="AT_sb", tag="mid")
            for ic in range(CT):
                a_ps = mmps.tile([128, C], F32, name="a_ps", tag="mm")
                for d in range(CT):
                    nc.tensor.matmul(
                        out=a_ps,
                        lhsT=wv_sb[:, d, ic * 128 : (ic + 1) * 128],
                        rhs=attnT_sb[:, d, :],
                        start=(d == 0),
                        stop=(d == CT - 1),
                    )
                copy_evac(ic + 1, AT_sb[:, ic, :], a_ps)
            st["AT"] = AT_sb
            # t = attn_un @ bv as per-partition dot products on DVE
            tts = vecp.tile([128, C], F32, name="tts", tag="tts", bufs=1)
            tcol = vecp.tile([128, CT], F32, name="tcol", tag="tcol")
            for cc in range(CT):
                nc.vector.tensor_mul(
                    tts, st["attn"][:, cc, :].bitcast(F32), bv_rep.bitcast(F32)
                )
                nc.vector.reduce_sum(out=tcol[:, cc : cc + 1], in_=tts, axis=AX.X)
            rt = vecp.tile([128, CT], F32, name="rt", tag="rt")
            nc.vector.tensor_mul(rt, tcol, st["recip"])
            st["rt"] = rt

        def out_phase(b, q, st):
            for cc in range(CT):
                stage = outp.tile([128, PBQ, 512], F32, name="stage", tag="stage")
                for pb in range(PBQ):
                    o_ps = ops.tile([128, 512], F32, name="o_ps", tag="out")
                    for i in range(CT):
                        nc.tensor.matmul(
                            out=o_ps,
                            lhsT=st["AT"][:, i, cc * 128 : (cc + 1) * 128],
                            rhs=st[f"x{q}"][:, i, pb * 512 : (pb + 1) * 512],
                            start=(i == 0),
                            stop=(i == CT - 1),
                        )
                    if pb % 2 == 0:
                        nc.scalar.activation(
                            out=stage[:, pb, :],
                            in_=o_ps,
                            func=ACTF.Identity,
                            bias=st["rt"][:, cc : cc + 1],
                            scale=st["recip"][:, cc : cc + 1],
                        )
                    else:
                        nc.vector.tensor_scalar(
                            out=stage[:, pb, :],
                            in0=o_ps,
                            scalar1=st["recip"][:, cc : cc + 1],
                            scalar2=st["rt"][:, cc : cc + 1],
                            op0=ALU.mult,
                            op1=ALU.add,
                        )
                nc.sync.dma_start(
                    out=out_d[
                        b, cc * 128 : (cc + 1) * 128, q * QTR : (q + 1) * QTR
                    ].rearrange("p (pb f) -> p pb f", f=512),
                    in_=stage,
                )
            del st[f"x{q}"]

        # ---- schedule: batch-1 PE work is threaded into batch-0's
        # copy/softmax latency windows (and vice versa) so the PE never
        # idles long enough for the HAM clock gate to re-throttle ----
        for q in range(NQ):
            sreduce(0, q, st0)
            transpose_G(0, q, st0)
        s_finish(0, st0)
        G_copy(0, st0)
        transpose_G(1, 0, st1)     # covers b0 G-mirror + U dependency stalls
        U_phase(0, st0)
        E_softmax(0, st0)
        transpose_G(1, 1, st1)     # covers b0 softmax
        attnT_AT(0, st0)
        out_phase(0, 0, st0)
        load_x(1, 2, st1)          # reuses freed x slot
        out_phase(0, 1, st0)
        load_x(1, 3, st1)
        out_phase(0, 2, st0)
        for q in range(NQ):        # ACT row-sums land in the b1 transpose
            sreduce(1, q, st1)     # windows, where ACT is otherwise idle
        transpose_G(1, 2, st1)
        transpose_G(1, 3, st1)
        s_finish(1, st1)
        G_copy(1, st1)
        U_phase(1, st1)
        E_softmax(1, st1)
        out_phase(0, 3, st0)       # covers b1 softmax
        attnT_AT(1, st1)
        out_phase(1, 0, st1)
        out_phase(1, 1, st1)
        out_phase(1, 2, st1)
        out_phase(1, 3, st1)

    nc.compile()
    return nc


_CACHE = {}


def _get_nc():
    if "nc" not in _CACHE:
        _CACHE["nc"] = build_nc()
    return _CACHE["nc"]


def make_in_maps(x, Wq, bq, Wk, bk, Wv, bv):
    x = np.ascontiguousarray(np.asarray(x, np.float32))
    Wq = np.asarray(Wq, np.float32)
    Wk = np.asarray(Wk, np.float32)
    Wv = np.ascontiguousarray(np.asarray(Wv, np.float32))
    bq = np.asarray(bq, np.float32)
    bk = np.asarray(bk, np.float32)
    bv = np.asarray(bv, np.float32)
    wqt = np.ascontiguousarray(Wq.T)
    wkt = np.ascontiguousarray(Wk.T)
    shared = {
        "wqt": wqt,
        "wkt": wkt,
        "wv": Wv,
        "bq_row": np.ascontiguousarray(bq[None, :]),
        "bk_row": np.ascontiguousarray(bk[None, :]),
        "pbk_row": np.ascontiguousarray((float(P) * bk)[None, :]),
        "bv_row": np.ascontiguousarray(bv[None, :]),
        "ident": np.eye(128, dtype=np.float32),
    }
    return [
        {"x": np.ascontiguousarray(x[BPC * i : BPC * (i + 1)]), **shared}
        for i in range(N_CORES)
    ]


def run(inputs, trace=False, tmpdir=None):
    nc = _get_nc()
    in_maps = make_in_maps(**inputs)
    res = run_bass_kernel_spmd(
        nc, in_maps, core_ids=list(range(N_CORES)), trace=trace, tmpdir=tmpdir
    )
    out = np.concatenate([res.results[i]["out"] for i in range(N_CORES)], axis=0)
    return out.astype(np.float32, copy=False), res


def kernel(**inputs) -> np.ndarray:
    out, _ = run(inputs, trace=False)
    return out



# revision 18
# speedup vs baseline: 1.2542x; 1.2542x over previous
"""Channel-attention (bmm-softmax-bmm over channels) on 8 TRN2 NeuronCores.

Math (per batch b):
    q = Wq x + bq 1^T ; k = Wk x + bk 1^T ; v = Wv x + bv 1^T      (x: [C, P])
    E = q k^T ; attn = softmax(E, axis=-1) ; out = attn v

Gram reformulation (cuts MACs ~2.6x):
    G = x x^T (symmetric: only upper-triangle block-row computed)
    s = x @ 1_P
    E = WqT.T @ (G WkT + s bk^T) + bq r^T,   r = Wk s + P bk
    attn_un = exp(E - rowmax), Z = rowsum(attn_un)
    AT = Wv^T attn_un^T ;  t = attn_un @ bv
    out = (AT.T @ x + t 1^T) * (1/Z) per-row

Sharding: data-parallel over B; core i gets batches [2i, 2i+1]; weights
replicated. No cross-core communication.

Precision plan (rel err ~7e-3 vs fp32 reference, gate 2e-2):
  - x fed twice from HBM in bf16: natural layout [C,P] for the output
    matmul and host-pretransposed [P,C] for the Gram (removes all PE
    transposes of x and their PSUM evacuations).
  - Gram + output matmuls in bf16 (FWL halves weight-load time).
  - Logit path (G_sb, U, WqT, WkT, bias rows) in float32r: softmax
    amplifies logit errors, bf16 there fails the gate.
  - attn / AT / Wv / staged output in bf16 (linear path, insensitive).
"""

import os
from contextlib import ExitStack

import numpy as np
from ml_dtypes import bfloat16

import concourse.bass as bass
from concourse import bacc
import concourse.mybir as mybir
import concourse.tile as tile
from concourse.bass_utils import run_bass_kernel_spmd

B, C, P = 16, 512, 4096
N_CORES = 8
BPC = B // N_CORES           # batches per core
CT = C // 128                # 4 c-tiles
QTR = 1024                   # x quarter width
NQ = P // QTR                # 4 quarters per batch
CHT = 8                      # p-tiles per xT chunk
NCH = P // (128 * CHT)       # 4 chunks
F32 = mybir.dt.float32
F32R = mybir.dt.float32r
BF16 = mybir.dt.bfloat16

AX = mybir.AxisListType
ALU = mybir.AluOpType
ACTF = mybir.ActivationFunctionType


def build_nc():
    nc = bacc.Bacc(trn_type="TRN2", target_bir_lowering=False, debug=False)

    x_d = nc.dram_tensor("x", [BPC, C, P], BF16, kind="ExternalInput")
    xt_d = nc.dram_tensor("xt", [BPC, P, C], BF16, kind="ExternalInput")
    wqt_d = nc.dram_tensor("wqt", [C, C], F32R, kind="ExternalInput")
    wkt_d = nc.dram_tensor("wkt", [C, C], F32R, kind="ExternalInput")
    wv_d = nc.dram_tensor("wv", [C, C], BF16, kind="ExternalInput")
    bqr_d = nc.dram_tensor("bq_row", [1, C], F32R, kind="ExternalInput")
    bkr_d = nc.dram_tensor("bk_row", [1, C], F32R, kind="ExternalInput")
    pbk_d = nc.dram_tensor("pbk_row", [1, C], F32R, kind="ExternalInput")
    bvr_d = nc.dram_tensor("bv_row", [1, C], BF16, kind="ExternalInput")
    identr_d = nc.dram_tensor("identr", [128, 128], F32R, kind="ExternalInput")
    identb_d = nc.dram_tensor("identb", [128, 128], BF16, kind="ExternalInput")
    out_d = nc.dram_tensor("out", [BPC, C, P], BF16, kind="ExternalOutput")

    DBG = bool(os.environ.get("KDBG"))
    if DBG:
        dbg_g = nc.dram_tensor("dbg_g", [BPC, 128, CT, C], F32R, kind="ExternalOutput")
        dbg_u = nc.dram_tensor("dbg_u", [BPC, 128, CT, C], F32R, kind="ExternalOutput")
        dbg_a = nc.dram_tensor("dbg_a", [BPC, 128, CT, C], BF16, kind="ExternalOutput")
        dbg_at = nc.dram_tensor("dbg_at", [BPC, 128, CT, C], BF16, kind="ExternalOutput")
        dbg_s = nc.dram_tensor("dbg_s", [BPC, 1, C], F32R, kind="ExternalOutput")
        dbg_r = nc.dram_tensor("dbg_r", [BPC, 1, C], F32R, kind="ExternalOutput")

    with ExitStack() as ctx:
        tc = ctx.enter_context(tile.TileContext(nc))
        const = ctx.enter_context(tc.tile_pool(name="const", bufs=1))
        xpool = ctx.enter_context(tc.tile_pool(name="xpool", bufs=8))
        xtp = ctx.enter_context(tc.tile_pool(name="xtp", bufs=4))
        gsbp = ctx.enter_context(tc.tile_pool(name="gsbp", bufs=2))
        usbp = ctx.enter_context(tc.tile_pool(name="usbp", bufs=2))
        atnp = ctx.enter_context(tc.tile_pool(name="atnp", bufs=2))
        atntp = ctx.enter_context(tc.tile_pool(name="atntp", bufs=2))
        atp = ctx.enter_context(tc.tile_pool(name="atp", bufs=2))
        vecp = ctx.enter_context(tc.tile_pool(name="vecp", bufs=2))
        outp = ctx.enter_context(tc.tile_pool(name="outp", bufs=4))
        gps = ctx.enter_context(tc.tile_pool(name="gps", bufs=1, space="PSUM"))
        ops = ctx.enter_context(tc.tile_pool(name="ops", bufs=2, space="PSUM"))
        mmps = ctx.enter_context(tc.tile_pool(name="mmps", bufs=2, space="PSUM"))

        st0, st1 = {}, {}

        # ---- DMA loads (sync queue, ordered by first need) ----
        def load_xt(b, ch, st, split=1):
            xtt = xtp.tile([128, CHT, C], BF16, name=f"xt_b{b}c{ch}", tag="xt")
            w = CHT // split
            for s in range(split):
                r0 = ch * CHT * 128 + s * w * 128
                nc.sync.dma_start(
                    out=xtt[:, s * w : (s + 1) * w, :],
                    in_=xt_d[b, r0 : r0 + w * 128, :].rearrange(
                        "(n p) c -> p n c", p=128
                    ),
                )
            st[f"xt{ch}"] = xtt

        def load_x(b, q, st):
            xt_ = xpool.tile([128, CT, QTR], BF16, name=f"x_b{b}q{q}", tag="x")
            nc.sync.dma_start(
                out=xt_,
                in_=x_d[b, :, q * QTR : (q + 1) * QTR].rearrange(
                    "(t p) f -> p t f", p=128
                ),
            )
            st[f"x{q}"] = xt_

        load_xt(0, 0, st0, split=2)
        load_xt(0, 1, st0)
        load_xt(0, 2, st0)
        load_xt(0, 3, st0)

        identr = const.tile([128, 128], F32R, name="identr")
        nc.sync.dma_start(out=identr, in_=identr_d[:, :])
        identb = const.tile([128, 128], BF16, name="identb")
        nc.sync.dma_start(out=identb, in_=identb_d[:, :])

        ones11_f = const.tile([1, 1], F32, name="ones11_f")
        nc.vector.memset(ones11_f, 1.0)
        ones11 = const.tile([1, 1], F32R, name="ones11")
        nc.vector.tensor_copy(ones11, ones11_f)

        load_x(0, 0, st0)
        load_x(0, 1, st0)

        def load_w(name, d, dt):
            t = const.tile([128, CT, C], dt, name=name)
            nc.sync.dma_start(out=t, in_=d[:, :].rearrange("(t p) f -> p t f", p=128))
            return t

        wkt_sb = load_w("wkt_sb", wkt_d, F32R)
        bkr_sb = const.tile([1, C], F32R, name="bkr_sb")
        nc.sync.dma_start(out=bkr_sb, in_=bkr_d[:, :])
        pbk_sb = const.tile([1, C], F32R, name="pbk_sb")
        nc.sync.dma_start(out=pbk_sb, in_=pbk_d[:, :])

        load_x(0, 2, st0)
        load_x(0, 3, st0)

        wqt_sb = load_w("wqt_sb", wqt_d, F32R)
        bqr_sb = const.tile([1, C], F32R, name="bqr_sb")
        nc.sync.dma_start(out=bqr_sb, in_=bqr_d[:, :])
        wv_sb = load_w("wv_sb", wv_d, BF16)
        bv_rep = const.tile([128, C], BF16, name="bv_rep")
        nc.sync.dma_start(out=bv_rep, in_=bvr_d[:, :].partition_broadcast(128))

        # ---- per-batch phases ----
        sdump = const.tile([128, QTR], BF16, name="sdump")

        def sreduce(b, q, st):
            # row-sums on ACT: Identity activation with fp32 accumulate.
            sc = vecp.tile([128, CT], F32, name=f"scol_b{b}q{q}", tag=f"scol_q{q}")
            for t in range(CT):
                nc.scalar.activation(
                    out=sdump,
                    in_=st[f"x{q}"][:, t, :],
                    func=ACTF.Identity,
                    accum_out=sc[:, t : t + 1],
                )
            st[f"scol_q{q}"] = sc

        def G_mms(b, ch, st):
            """Gram accumulation for one xT chunk (8 p-tiles).  Upper
            triangle block-rows only; cc=2/3 share one PSUM bank."""
            if ch == 0:
                # one FULL bank per accumulation group: start=True clears
                # has_written at bank granularity, so two concurrent groups
                # must never share a bank (measured: the second group's
                # start wipes the first's initial contribution).
                st["G_ps"] = [
                    gps.tile([128, 512], F32, name=f"G{cc}_b{b}", tag=f"G{cc}")
                    for cc in range(CT)
                ]
            outs = [st["G_ps"][cc][:, : 512 - cc * 128] for cc in range(CT)]
            xtt = st[f"xt{ch}"]
            for n in range(CHT):
                first = ch == 0 and n == 0
                last = ch == NCH - 1 and n == CHT - 1
                for cc in range(CT):
                    nc.tensor.matmul(
                        out=outs[cc],
                        lhsT=xtt[:, n, cc * 128 : (cc + 1) * 128],
                        rhs=xtt[:, n, cc * 128 :],
                        start=first,
                        stop=last,
                    )

        def s_finish(b, st):
            sa = vecp.tile([128, CT], F32, name="sa", tag="sa")
            sb_ = vecp.tile([128, CT], F32, name="sb_", tag="sb_")
            nc.vector.tensor_add(sa, st["scol_q0"], st["scol_q1"])
            nc.vector.tensor_add(sb_, st["scol_q2"], st["scol_q3"])
            scol = vecp.tile([128, CT], F32R, name="scol", tag="scol")
            nc.vector.tensor_add(scol, sa, sb_)
            st["scol"] = scol
            srow_ps = mmps.tile([1, C], F32R, name="srow_ps", tag="mm")
            for t in range(CT):
                nc.tensor.transpose(
                    out=srow_ps[:, t * 128 : (t + 1) * 128],
                    in_=scol[:, t : t + 1],
                    identity=identr,
                )
            srow = vecp.tile([1, C], F32R, name="srow", tag="srow", bufs=1)
            nc.vector.tensor_copy(srow, srow_ps)
            st["srow"] = srow
            if DBG:
                nc.sync.dma_start(out=dbg_s[b], in_=srow)

        def G_evac(b, st):
            """Evacuate the upper-triangle block-row of G to SBUF (DVE)."""
            G_sb = gsbp.tile([128, CT, C], F32R, name="G_sb", tag="gsb")
            for cc in range(CT):
                nc.vector.tensor_copy(
                    G_sb[:, cc, cc * 128 : 512], st["G_ps"][cc][:, : 512 - cc * 128]
                )
            st["G_sb"] = G_sb
            del st["G_ps"]

        def G_mirror(b, st):
            """Mirror strictly-lower blocks via PE transposes (G symmetric)."""
            G_sb = st["G_sb"]
            pairs = [(dd, cc) for cc in range(CT) for dd in range(cc)]
            lps = [
                mmps.tile([128, 512], F32R, name=f"lps{i}", tag="mm")
                for i in range(2)
            ]
            for i, (dd, cc) in enumerate(pairs):
                nc.tensor.transpose(
                    out=lps[i // 3][:, (i % 3) * 128 : (i % 3 + 1) * 128],
                    in_=G_sb[:, dd, cc * 128 : (cc + 1) * 128],
                    identity=identr,
                )
            for i, (dd, cc) in enumerate(pairs):
                nc.vector.tensor_copy(
                    G_sb[:, cc, dd * 128 : (dd + 1) * 128],
                    lps[i // 3][:, (i % 3) * 128 : (i % 3 + 1) * 128],
                )
            if DBG:
                nc.sync.dma_start(out=dbg_g[b], in_=G_sb)

        def U_phase(b, st):
            U_sb = usbp.tile([128, CT, C], F32R, name="U_sb", tag="usb")
            for ic in range(CT):
                u_ps = ops.tile([128, C], F32, name="u_ps", tag="out")
                for e in range(CT):
                    nc.tensor.matmul(
                        out=u_ps,
                        lhsT=st["G_sb"][:, e, ic * 128 : (ic + 1) * 128],
                        rhs=wkt_sb[:, e, :],
                        start=(e == 0),
                        stop=False,
                    )
                nc.tensor.matmul(
                    out=u_ps,
                    lhsT=st["srow"][:, ic * 128 : (ic + 1) * 128],
                    rhs=bkr_sb,
                    start=False,
                    stop=True,
                )
                nc.scalar.copy(U_sb[:, ic, :], u_ps)
            st["U_sb"] = U_sb
            # r = Wk s + P bk   (as a row [1, C])
            r_ps = mmps.tile([1, C], F32, name="r_ps", tag="mm")
            for t in range(CT):
                nc.tensor.matmul(
                    out=r_ps,
                    lhsT=st["scol"][:, t : t + 1],
                    rhs=wkt_sb[:, t, :],
                    start=(t == 0),
                    stop=False,
                )
            nc.tensor.matmul(
                out=r_ps, lhsT=ones11, rhs=pbk_sb, start=False, stop=True
            )
            rrow = vecp.tile([1, C], F32R, name="rrow", tag="rrow", bufs=1)
            nc.vector.tensor_copy(rrow, r_ps)
            st["rrow"] = rrow
            if DBG:
                nc.sync.dma_start(out=dbg_u[b], in_=U_sb)
                nc.sync.dma_start(out=dbg_r[b], in_=rrow)

        def E_softmax(b, st):
            """E matmuls; softmax reads the PSUM bank directly."""
            attn_sb = atnp.tile([128, CT, C], BF16, name="attn_sb", tag="atn")
            mx = vecp.tile([128, CT], F32, name="mx", tag="mx")
            negm = vecp.tile([128, CT], F32, name="negm", tag="negm")
            zsum = vecp.tile([128, CT], F32, name="zsum", tag="zsum")
            recip = vecp.tile([128, CT], F32, name="recip", tag="recip")
            for cc in range(CT):
                e_ps = ops.tile([128, C], F32, name="e_ps", tag="out")
                for i in range(CT):
                    nc.tensor.matmul(
                        out=e_ps,
                        lhsT=wqt_sb[:, i, cc * 128 : (cc + 1) * 128],
                        rhs=st["U_sb"][:, i, :],
                        start=(i == 0),
                        stop=False,
                    )
                nc.tensor.matmul(
                    out=e_ps,
                    lhsT=bqr_sb[:, cc * 128 : (cc + 1) * 128],
                    rhs=st["rrow"],
                    start=False,
                    stop=True,
                )
                nc.vector.reduce_max(
                    out=mx[:, cc : cc + 1], in_=e_ps, axis=AX.X
                )
                nc.vector.tensor_scalar_mul(
                    negm[:, cc : cc + 1], mx[:, cc : cc + 1], -1.0
                )
                nc.scalar.activation(
                    out=attn_sb[:, cc, :],
                    in_=e_ps,
                    func=ACTF.Exp,
                    bias=negm[:, cc : cc + 1],
                    scale=1.0,
                    accum_out=zsum[:, cc : cc + 1],
                )
            nc.vector.reciprocal(out=recip, in_=zsum)
            st["attn"] = attn_sb
            st["recip"] = recip
            if DBG:
                nc.sync.dma_start(out=dbg_a[b], in_=attn_sb)

        def attnT_AT(b, st):
            attnT_sb = atntp.tile([128, CT, C], BF16, name="attnT_sb", tag="atnt")
            for dc in range(CT):
                at_ps = mmps.tile([128, C], BF16, name="at_ps", tag="mm")
                for t in range(CT):
                    nc.tensor.transpose(
                        out=at_ps[:, t * 128 : (t + 1) * 128],
                        in_=st["attn"][:, t, dc * 128 : (dc + 1) * 128],
                        identity=identb,
                    )
                nc.vector.tensor_copy(attnT_sb[:, dc, :], at_ps)
            AT_sb = atp.tile([128, CT, C], BF16, name="AT_sb", tag="at")
            for ic in range(CT):
                a_ps = ops.tile([128, C], F32, name="a_ps", tag="out")
                for d in range(CT):
                    nc.tensor.matmul(
                        out=a_ps,
                        lhsT=wv_sb[:, d, ic * 128 : (ic + 1) * 128],
                        rhs=attnT_sb[:, d, :],
                        start=(d == 0),
                        stop=(d == CT - 1),
                    )
                nc.scalar.copy(AT_sb[:, ic, :], a_ps)
            st["AT"] = AT_sb
            # t = attn_un @ bv as per-partition dot products on DVE
            tts = vecp.tile([128, C], F32, name="tts", tag="tts", bufs=1)
            tcol = vecp.tile([128, CT], F32, name="tcol", tag="tcol")
            for cc in range(CT):
                nc.vector.tensor_mul(tts, st["attn"][:, cc, :], bv_rep)
                nc.vector.reduce_sum(out=tcol[:, cc : cc + 1], in_=tts, axis=AX.X)
            rt = vecp.tile([128, CT], F32, name="rt", tag="rt")
            nc.vector.tensor_mul(rt, tcol, st["recip"])
            st["rt"] = rt
            if DBG:
                nc.sync.dma_start(out=dbg_at[b], in_=AT_sb)

        def out_phase(b, q, st):
            for cc in range(CT):
                stage = outp.tile([128, QTR], BF16, name="stage", tag="stage")
                for pb in range(2):
                    o_ps = ops.tile([128, 512], F32, name="o_ps", tag="out")
                    for i in range(CT):
                        nc.tensor.matmul(
                            out=o_ps,
                            lhsT=st["AT"][:, i, cc * 128 : (cc + 1) * 128],
                            rhs=st[f"x{q}"][:, i, pb * 512 : (pb + 1) * 512],
                            start=(i == 0),
                            stop=(i == CT - 1),
                        )
                    if pb % 2 == 0:
                        nc.scalar.activation(
                            out=stage[:, pb * 512 : (pb + 1) * 512],
                            in_=o_ps,
                            func=ACTF.Identity,
                            bias=st["rt"][:, cc : cc + 1],
                            scale=st["recip"][:, cc : cc + 1],
                        )
                    else:
                        nc.vector.tensor_scalar(
                            out=stage[:, pb * 512 : (pb + 1) * 512],
                            in0=o_ps,
                            scalar1=st["recip"][:, cc : cc + 1],
                            scalar2=st["rt"][:, cc : cc + 1],
                            op0=ALU.mult,
                            op1=ALU.add,
                        )
                nc.sync.dma_start(
                    out=out_d[
                        b, cc * 128 : (cc + 1) * 128, q * QTR : (q + 1) * QTR
                    ],
                    in_=stage,
                )

        # ---- schedule: batch-1 Gram chunks are threaded into batch-0's
        # evac/softmax latency windows so the PE never idles long enough
        # for the HAM clock gate to re-throttle ----
        for ch in range(NCH):
            G_mms(0, ch, st0)
            # b1's chunk reuses b0-chunk's SBUF slot: the pool-ring WAR edge
            # only exists if the previous occupant's readers are already
            # emitted, so this load must follow G_mms(0, ch).
            load_xt(1, ch, st1)
        for q in range(NQ):
            load_x(1, q, st1)
        for q in range(NQ):
            sreduce(0, q, st0)
        G_evac(0, st0)
        G_mirror(0, st0)
        s_finish(0, st0)
        U_phase(0, st0)
        E_softmax(0, st0)
        G_mms(1, 0, st1)           # covers b0 softmax latency
        attnT_AT(0, st0)
        G_mms(1, 1, st1)           # covers AT evac + t/recip latency
        out_phase(0, 0, st0)
        G_mms(1, 2, st1)
        out_phase(0, 1, st0)
        G_mms(1, 3, st1)
        for q in range(NQ):
            sreduce(1, q, st1)
        out_phase(0, 2, st0)
        G_evac(1, st1)
        G_mirror(1, st1)
        s_finish(1, st1)
        U_phase(1, st1)
        E_softmax(1, st1)
        out_phase(0, 3, st0)       # covers b1 softmax latency
        attnT_AT(1, st1)
        out_phase(1, 0, st1)
        out_phase(1, 1, st1)
        out_phase(1, 2, st1)
        out_phase(1, 3, st1)

    nc.compile()
    return nc


_CACHE = {}


def _get_nc():
    if "nc" not in _CACHE:
        _CACHE["nc"] = build_nc()
    return _CACHE["nc"]


def make_in_maps(x, Wq, bq, Wk, bk, Wv, bv):
    x = np.asarray(x, np.float32)
    x_bf = x.astype(bfloat16)
    xt_bf = np.ascontiguousarray(x_bf.transpose(0, 2, 1))
    Wq = np.asarray(Wq, np.float32)
    Wk = np.asarray(Wk, np.float32)
    Wv = np.ascontiguousarray(np.asarray(Wv, np.float32).astype(bfloat16))
    bq = np.asarray(bq, np.float32)
    bk = np.asarray(bk, np.float32)
    bv = np.asarray(bv, np.float32).astype(bfloat16)
    shared = {
        "wqt": np.ascontiguousarray(Wq.T),
        "wkt": np.ascontiguousarray(Wk.T),
        "wv": Wv,
        "bq_row": np.ascontiguousarray(bq[None, :]),
        "bk_row": np.ascontiguousarray(bk[None, :]),
        "pbk_row": np.ascontiguousarray((float(P) * bk)[None, :]),
        "bv_row": np.ascontiguousarray(bv[None, :]),
        "identr": np.eye(128, dtype=np.float32),
        "identb": np.eye(128, dtype=bfloat16),
    }
    return [
        {
            "x": np.ascontiguousarray(x_bf[BPC * i : BPC * (i + 1)]),
            "xt": np.ascontiguousarray(xt_bf[BPC * i : BPC * (i + 1)]),
            **shared,
        }
        for i in range(N_CORES)
    ]


def run(inputs, trace=False, tmpdir=None):
    nc = _get_nc()
    in_maps = make_in_maps(**inputs)
    res = run_bass_kernel_spmd(
        nc, in_maps, core_ids=list(range(N_CORES)), trace=trace, tmpdir=tmpdir
    )
    out = np.concatenate([res.results[i]["out"] for i in range(N_CORES)], axis=0)
    return out.astype(np.float32), res


def kernel(**inputs) -> np.ndarray:
    out, _ = run(inputs, trace=False)
    return out
